# revision 7
# baseline (speedup 1.0000x reference)
"""GQA attention kernel for 8 Trainium2 NeuronCores.

Sharding: 8 cores = 2 (batch) x 4 (kv-head groups). Each core computes, for its
(b, g): q/k/v projections for 4 query heads + 1 kv head, RoPE, causal
flash-style attention entirely on-chip, and a row-sharded o_proj partial
y_partial = att @ Wo[g-rows]. Host sums the 4 partials per batch.

Device layout tricks:
  - x is shipped pre-transposed (xT [D, T]) so projections need no on-chip
    transpose: qT/kT come out head-dim-major, v comes out token-major.
  - Wq/Wk columns are permuted per head to [even dims | odd dims] so RoPE is
    two contiguous [32, T] halves (permutation cancels in q.k).
  - Scores are computed transposed (S^T [tk, tq]): the AV matmul then
    contracts over tk on partitions (full PE rows) and the softmax denominator
    comes free from a ones-column appended to V (M=65).
  - kT is written twice (partitions 0-63 and 64-127) so the two q-head-pair
    score matmuls (K=64) land on disjoint PE row-groups and run concurrently.
  - Causality at [tk=128, tq=512] block granularity: blocks above the diagonal
    are skipped, the 4 diagonal-straddling block shapes get a 0/1 mask multiply.
"""
import math

import numpy as np
import ml_dtypes

import concourse.bass as bass
import concourse.bacc as bacc
import concourse.mybir as mybir
import concourse.tile as tile
from concourse.bass import ds, ts
from concourse.bass_utils import run_bass_kernel_spmd

B, T, D = 2, 2048, 1024
H, KV, DH = 16, 4, 64
HL = H // KV          # 4 local query heads per core
QM = HL * DH // 128   # 2 q-head-pair M-tiles
NT = T // 512         # 4 tq tiles of 512
TK = T // 128         # 16 tk tiles of 128
KD = D // 128         # 8 contraction chunks
ROPE_THETA = 500000.0
SCALE = 1.0 / math.sqrt(DH)

F32 = mybir.dt.float32
BF16 = mybir.dt.bfloat16
BF = ml_dtypes.bfloat16


def _build_body(tc):
    nc = tc.nc
    xt_d = nc.dram_tensor("xt", [D, T], BF16, kind="ExternalInput")
    wq_d = nc.dram_tensor("wq", [D, HL * DH], BF16, kind="ExternalInput")
    wk_d = nc.dram_tensor("wk", [D, DH], BF16, kind="ExternalInput")
    wv_d = nc.dram_tensor("wv", [D, DH], BF16, kind="ExternalInput")
    wo_d = nc.dram_tensor("wo", [HL * DH, D], BF16, kind="ExternalInput")
    cos_d = nc.dram_tensor("cosT", [128, T], F32, kind="ExternalInput")
    sin_d = nc.dram_tensor("sinT", [128, T], F32, kind="ExternalInput")
    msk_d = nc.dram_tensor("dmask", [128, 4 * 512], BF16, kind="ExternalInput")
    out_d = nc.dram_tensor("out", [T, D], F32, kind="ExternalOutput")

    with (
        tc.tile_pool(name="cst", bufs=1) as cst,
        tc.tile_pool(name="pp", bufs=2, space="PSUM") as pp,
        tc.tile_pool(name="sp", bufs=2, space="PSUM") as sp,
        tc.tile_pool(name="avp", bufs=2, space="PSUM") as avp,
        tc.tile_pool(name="bcp", bufs=1, space="PSUM") as bcp,
        tc.tile_pool(name="ypp", bufs=1, space="PSUM") as ypp,
        tc.tile_pool(name="rtp", bufs=8) as rtp,
        tc.tile_pool(name="esp", bufs=24) as esp,
        tc.tile_pool(name="ysp", bufs=3) as ysp,
        tc.tile_pool(name="rcp", bufs=2) as rcp,
        tc.tile_pool(name="bsp", bufs=2) as bsp,
    ):
        # persistent SBUF tensors (2D: [partitions, flattened blocks])
        xt = cst.tile([128, KD * T], BF16, tag="xt")          # [p, k*T + t]
        wq = cst.tile([128, KD * HL * DH], BF16, tag="wq")    # [p, k*256 + m]
        wk = cst.tile([128, KD * DH], BF16, tag="wk")
        wv = cst.tile([128, KD * DH], BF16, tag="wv")
        wo = cst.tile([128, 2 * D], BF16, tag="wo")           # [p, kc*D + d]
        cosb = cst.tile([128, T], F32, tag="cos")             # 4x replicated
        sinb = cst.tile([128, T], F32, tag="sin")
        msk = cst.tile([128, 4 * 512], BF16, tag="msk")
        ones = cst.tile([1, 64], BF16, tag="ones")
        qt = cst.tile([128, QM * T], BF16, tag="qt")          # [p, m*T + t]
        kt = cst.tile([128, T], BF16, tag="kt")               # rows 64-127 dup
        vt = cst.tile([128, TK * (DH + 1)], BF16, tag="vt")   # [p, j*65 + dh]
        att = cst.tile([128, QM * T], BF16, tag="att")

        # ---- loads ----
        for k in range(KD):
            nc.sync.dma_start(wq[:, ds(k * HL * DH, HL * DH)],
                              wq_d[ds(k * 128, 128), :])
            nc.sync.dma_start(wk[:, ds(k * DH, DH)], wk_d[ds(k * 128, 128), :])
            nc.sync.dma_start(wv[:, ds(k * DH, DH)], wv_d[ds(k * 128, 128), :])
        for kc in range(2):
            nc.sync.dma_start(wo[:, ds(kc * D, D)], wo_d[ds(kc * 128, 128), :])
        nc.sync.dma_start(cosb[:], cos_d[:])
        nc.sync.dma_start(sinb[:], sin_d[:])
        nc.sync.dma_start(msk[:], msk_d[:])
        nc.vector.memset(ones[:], 1.0)
        for j in range(TK):
            nc.vector.memset(vt[:, ds(j * (DH + 1) + DH, 1)], 1.0)
        # x^T, column-block-major so early tq/tk tiles arrive first
        for n in range(NT):
            for k in range(KD):
                nc.sync.dma_start(xt[:, ds(k * T + n * 512, 512)],
                                  xt_d[ds(k * 128, 128), ds(n * 512, 512)])

        def rope32(dst, dst_row, dst_col, src, e_row, n):
            """dst rows [dst_row, dst_row+32)+[.. +64) <- roped src halves."""
            e = src[ds(e_row, 32), :]
            o = src[ds(e_row + 32, 32), :]
            c = cosb[ds(e_row, 32), ds(n * 512, 512)]
            s = sinb[ds(e_row, 32), ds(n * 512, 512)]
            t1 = rtp.tile([32, 512], F32, tag="rt")
            t2 = rtp.tile([32, 512], F32, tag="rt")
            nc.vector.tensor_mul(t1[:], e, c)
            nc.vector.tensor_mul(t2[:], o, s)
            nc.vector.tensor_sub(dst[ds(dst_row, 32), ds(dst_col, 512)],
                                 t1[:], t2[:])
            t3 = rtp.tile([32, 512], F32, tag="rt")
            t4 = rtp.tile([32, 512], F32, tag="rt")
            nc.vector.tensor_mul(t3[:], o, c)
            nc.vector.tensor_mul(t4[:], e, s)
            nc.vector.tensor_add(dst[ds(dst_row + 32, 32), ds(dst_col, 512)],
                                 t3[:], t4[:])

        # ---- q projection + rope ----
        for m in range(QM):
            for n in range(NT):
                ps = pp.tile([128, 512], F32, tag="pp")
                for k in range(KD):
                    nc.tensor.matmul(
                        ps[:],
                        wq[:, ds(k * HL * DH + m * 128, 128)],
                        xt[:, ds(k * T + n * 512, 512)],
                        start=(k == 0), stop=(k == KD - 1))
                for h2 in range(2):
                    rope32(qt, h2 * 64, m * T + n * 512, ps, h2 * 64, n)

        # ---- k projection + rope (written twice for PE row-tiling) ----
        for n in range(NT):
            ps = pp.tile([128, 512], F32, tag="pp")
            for k in range(KD):
                nc.tensor.matmul(
                    ps[ds(0, 64), :],
                    wk[:, ds(k * DH, DH)],
                    xt[:, ds(k * T + n * 512, 512)],
                    start=(k == 0), stop=(k == KD - 1))
            rope32(kt, 0, n * 512, ps, 0, n)
            rope32(kt, 64, n * 512, ps, 0, n)

        # ---- v projection (token-major, with appended ones column) ----
        for j in range(TK):
            psv = ypp.tile([128, 512], F32, tag="ypp")
            for k in range(KD):
                nc.tensor.matmul(
                    psv[:, ds(0, 64)],
                    xt[:, ds(k * T + j * 128, 128)],
                    wv[:, ds(k * DH, DH)],
                    start=(k == 0), stop=(k == KD - 1))
            nc.scalar.copy(vt[:, ds(j * (DH + 1), DH)], psv[:, ds(0, 64)])

        # ---- attention: S^T blocks -> exp -> AV with fused denom ----
        for m in range(QM):
            for i in range(NT):
                ntk = 4 * (i + 1)
                es_lists = ([], [])
                for j in range(ntk):
                    for h2 in range(2):
                        q_ap = qt[ds(h2 * 64, 64), ds(m * T + i * 512, 512)]
                        sps = sp.tile([128, 512], F32, tag="sp")
                        nc.tensor.matmul(sps[:],
                                         kt[ds(h2 * 64, 64), ds(j * 128, 128)],
                                         q_ap, start=True, stop=True)
                        es = esp.tile([128, 512], BF16, tag="es")
                        nc.scalar.activation(es[:], sps[:],
                                             mybir.ActivationFunctionType.Exp,
                                             scale=SCALE)
                        delta = j * 128 - i * 512
                        if delta >= 0:
                            nc.vector.tensor_mul(
                                es[:], es[:],
                                msk[:, ds((delta // 128) * 512, 512)])
                        es_lists[h2].append(es)
                for h2 in range(2):
                    av = avp.tile([65, 512], F32, tag="avp")
                    for j, es in enumerate(es_lists[h2]):
                        nc.tensor.matmul(av[:],
                                         vt[:, ds(j * (DH + 1), DH + 1)],
                                         es[:], start=(j == 0),
                                         stop=(j == ntk - 1))
                    rec = rcp.tile([1, 512], F32, tag="rec")
                    nc.vector.reciprocal(rec[:], av[ds(64, 1), :])
                    recb = rcp.tile([1, 512], BF16, tag="recb")
                    nc.vector.tensor_copy(recb[:], rec[:])
                    bc = bcp.tile([64, 512], F32, tag="bcp")
                    nc.tensor.matmul(bc[:], ones[:], recb[:],
                                     start=True, stop=True)
                    bcs = bsp.tile([64, 512], F32, tag="bcs")
                    nc.scalar.copy(bcs[:], bc[:])
                    for half in range(2):
                        nc.vector.tensor_mul(
                            att[ds(h2 * 64 + half * 32, 32),
                                ds(m * T + i * 512, 512)],
                            av[ds(half * 32, 32), :],
                            bcs[ds(half * 32, 32), :])

        # ---- o_proj partial: y = att @ Wo_g ----
        for tq in range(TK):
            for dn in range(2):
                yp = ypp.tile([128, 512], F32, tag="ypp")
                for kc in range(2):
                    nc.tensor.matmul(
                        yp[:],
                        att[:, ds(kc * T + tq * 128, 128)],
                        wo[:, ds(kc * D + dn * 512, 512)],
                        start=(kc == 0), stop=(kc == 1))
                ysb = ysp.tile([128, 512], F32, tag="ysb")
                nc.vector.tensor_copy(ysb[:], yp[:])
                nc.sync.dma_start(out_d[ds(tq * 128, 128), ds(dn * 512, 512)],
                                  ysb[:])


_CACHE = {}


def _get_program():
    if "nc" not in _CACHE:
        nc = bacc.Bacc("TRN2", target_bir_lowering=False, debug=False,
                       num_devices=8)
        with tile.TileContext(nc) as tc:
            _build_body(tc)
        nc.compile()
        _CACHE["nc"] = nc
    return _CACHE["nc"]


def _host_tables():
    freqs = 1.0 / ROPE_THETA ** (np.arange(0, DH, 2, dtype=np.float32) / DH)
    ang = np.outer(np.arange(T, dtype=np.float32), freqs)
    cosT = np.ascontiguousarray(np.cos(ang).T.astype(np.float32))
    sinT = np.ascontiguousarray(np.sin(ang).T.astype(np.float32))
    cos4 = np.tile(cosT, (4, 1))  # [128, T] quadrant-replicated
    sin4 = np.tile(sinT, (4, 1))
    p = np.arange(128)[:, None]
    f = np.arange(512)[None, :]
    blocks = [(p + d * 128 <= f).astype(BF) for d in range(4)]
    dmask = np.concatenate(blocks, axis=1)
    return cos4, sin4, dmask


def make_in_maps(x, Wq, Wk, Wv, Wo):
    cos4, sin4, dmask = _host_tables()
    eo = np.concatenate([np.arange(0, DH, 2), np.arange(1, DH, 2)])
    in_maps = []
    for c in range(8):
        b, g = c // 4, c % 4
        qcols = np.concatenate([(g * HL + h) * DH + eo for h in range(HL)])
        worows = np.arange(g * HL * DH, (g + 1) * HL * DH)
        in_maps.append({
            "xt": np.ascontiguousarray(x[b].T).astype(BF),
            "wq": np.ascontiguousarray(Wq[:, qcols]).astype(BF),
            "wk": np.ascontiguousarray(Wk[:, g * DH + eo]).astype(BF),
            "wv": np.ascontiguousarray(Wv[:, g * DH:(g + 1) * DH]).astype(BF),
            "wo": np.ascontiguousarray(Wo[worows, :]).astype(BF),
            "cosT": cos4,
            "sinT": sin4,
            "dmask": dmask,
        })
    return in_maps


def run(x, Wq, Wk, Wv, Wo, trace=False, tmpdir=None):
    nc = _get_program()
    in_maps = make_in_maps(x, Wq, Wk, Wv, Wo)
    res = run_bass_kernel_spmd(nc, in_maps, list(range(8)), trace=trace,
                               tmpdir=tmpdir)
    out = np.zeros((B, T, D), dtype=np.float32)
    for c in range(8):
        out[c // 4] += res.results[c]["out"]
    return out, res


def kernel(x, mask, Wq, Wk, Wv, Wo):
    x = np.asarray(x, dtype=np.float32)
    out, _ = run(x, np.asarray(Wq, dtype=np.float32),
                 np.asarray(Wk, dtype=np.float32),
                 np.asarray(Wv, dtype=np.float32),
                 np.asarray(Wo, dtype=np.float32))
    return out
